# revision 47
# baseline (speedup 1.0000x reference)
"""Causal multi-head attention (B=2, L=2048, D=2048, H=16, rope theta=5e5)
on 8 Trainium2 NeuronCores.

Sharding: core c handles batch b = c//4 and heads 4*(c%4) .. 4*(c%4)+3.
Each core computes q/k/v projections for its 4 heads, rope, causal
attention, and a partial output projection y_part = attn_out @ Wo[rows].
Host sums the 4 partials per batch (row-parallel all-reduce) to produce y.

All matmuls run in float32r (TF32-like, 1 cycle/row on the PE) with fp32
accumulation. Softmax skips the max-subtraction (scores are ~N(0,1);
max over 134M samples < 7, exp is safe in fp32).
"""
import os
import sys

sys.path.insert(0, "/opt/trn_rl_repo")

import contextlib

import numpy as np

import concourse.bass as bass
import concourse.tile as tile
from concourse import bacc, mybir
from concourse.bass_utils import run_bass_kernel_spmd

F32 = mybir.dt.float32
F32R = mybir.dt.float32r
EXP = mybir.ActivationFunctionType.Exp

P = 128          # partitions / head dim / tile edge
L = 2048         # sequence length
D = 2048         # model dim
NH = 4           # heads per core
BL = 512         # query block (l-block) size
NB = L // BL     # 4 l-blocks
ND = D // P      # 16 d chunks
NDG = 4          # d chunks per weight DMA group
NLT = BL // P    # 4 l-tiles per block
THETA = 500000.0
NEG = -1.0e30

_SWAP = []
for _i in range(16):
    _SWAP += [2 * _i + 1, 2 * _i]


def build_program(nrep=1):
    nc = bacc.Bacc("TRN2", target_bir_lowering=False, debug=False, num_devices=8)

    x_d = nc.dram_tensor("x", [L, D], F32R, kind="ExternalInput").ap()
    wq_d = nc.dram_tensor("wq", [D, NH * P], F32R, kind="ExternalInput").ap()
    wk_d = nc.dram_tensor("wk", [D, NH * P], F32R, kind="ExternalInput").ap()
    wv_d = nc.dram_tensor("wv", [D, NH * P], F32R, kind="ExternalInput").ap()
    # wo pre-permuted on host to [P, NH, D]: wo_r[p, h, :] = Wo[h*P + p, :]
    wo_d = nc.dram_tensor("wo", [P, NH, D], F32R, kind="ExternalInput").ap()
    cos_d = nc.dram_tensor("cosT", [P, L], F32, kind="ExternalInput").ap()
    sig_d = nc.dram_tensor("sigT", [P, L], F32, kind="ExternalInput").ap()
    mask_d = nc.dram_tensor("masks", [P, 5 * P], F32R, kind="ExternalInput").ap()
    id_d = nc.dram_tensor("iden", [P, P], F32R, kind="ExternalInput").ap()
    ones_d = nc.dram_tensor("ones", [P, 1], F32R, kind="ExternalInput").ap()
    y_d = nc.dram_tensor("y", [L, D], F32, kind="ExternalOutput").ap()

    with tile.TileContext(nc) as tc:
        with (
            tc.tile_pool(name="const", bufs=1) as cpool,
            tc.tile_pool(name="persist", bufs=1) as ppool,
            tc.tile_pool(name="work", bufs=2) as wpool,
            tc.tile_pool(name="ps", bufs=8, space="PSUM") as ps,
        ):
            t_id = cpool.tile([P, P], F32R, tag="t_id")
            t_ones = cpool.tile([P, 1], F32R, tag="t_ones")
            t_mask = cpool.tile([P, 5 * P], F32R, tag="t_mask")
            nc.sync.dma_start(t_id[:], id_d[:])
            nc.sync.dma_start(t_ones[:], ones_d[:])
            nc.sync.dma_start(t_mask[:], mask_d[:])

            # persistent full-L tensors (per head slices along free axis)
            kT = ppool.tile([P, NH * L], F32R, tag="kT")     # [dh, h*L + l]
            vv = ppool.tile([P, ND * NH * P], F32R, tag="vv")  # [l%128, j0*512 + h*128 + dh]
            qTs = ppool.tile([P, NH * BL], F32R, tag="qTs")  # current block [dh, h*BL + l]

            rep_ctx = tc.For_i(0, nrep, 1) if nrep > 1 else contextlib.nullcontext()
            with rep_ctx:
                _build_body(nc, tc, locals())
    nc.compile()
    return nc


def _build_body(nc, tc, env):
    t_id, t_ones, t_mask = env["t_id"], env["t_ones"], env["t_mask"]
    kT, vv, qTs = env["kT"], env["vv"], env["qTs"]
    wpool, ps = env["wpool"], env["ps"]
    x_d, wq_d, wk_d, wv_d, wo_d = (
        env["x_d"], env["wq_d"], env["wk_d"], env["wv_d"], env["wo_d"]
    )
    cos_d, sig_d, y_d = env["cos_d"], env["sig_d"], env["y_d"]

    def emit_transpose(B):
        """x[B-block rows] -> xT (PE transposes).  Emitted one block ahead so
        the x DMAs get a phase of lead time and the PE transposes fill the
        softmax-denominator stall after attention."""
        l0 = B * BL
        xT = wpool.tile([P, ND * BL], F32R, tag="xT", bufs=1, name=f"xT{B}")
        xTv = xT[:].rearrange("p (dd c) -> p dd c", c=BL)
        for lt in range(NLT):
            xr = wpool.tile([P, D], F32R, tag="xr", bufs=3, name=f"xr{B}{lt}")
            if B == 0 and lt == 0:
                for q4 in range(4):
                    nc.sync.dma_start(
                        xr[:, q4 * BL : (q4 + 1) * BL],
                        x_d[l0 : l0 + P, q4 * BL : (q4 + 1) * BL],
                    )
            else:
                nc.sync.dma_start(xr[:], x_d[l0 + lt * P : l0 + (lt + 1) * P, :])
            for dg in range(4):
                pt = ps.tile([P, 4 * P], F32R, tag="ps")
                for i in range(4):
                    d = dg * 4 + i
                    nc.tensor.transpose(
                        pt[:, i * P : (i + 1) * P],
                        xr[:, d * P : (d + 1) * P],
                        t_id[:],
                    )
                nc.scalar.copy(
                    xTv[:, dg * 4 : (dg + 1) * 4, lt * P : (lt + 1) * P],
                    pt[:].rearrange("p (i c) -> p i c", c=P),
                )
        return xT

    xT_next = emit_transpose(0)
    for B in range(NB):
        l0 = B * BL
        xT = xT_next
        # rope tables for this block (gpsimd queue: keep SP free for weights)
        t_cos = wpool.tile([P, BL], F32, tag="t_cos", bufs=1)
        t_sig = wpool.tile([P, BL], F32, tag="t_sig", bufs=1)
        nc.gpsimd.dma_start(t_cos[:], cos_d[:, l0 : l0 + BL])
        nc.gpsimd.dma_start(t_sig[:], sig_d[:, l0 : l0 + BL])

        # ---- q / k projections + rope ----
        for which, w_dram in (("q", wq_d), ("k", wk_d)):
            psqk = [
                ps.tile([P, BL], F32, tag="ps", name=f"psqk{B}{which}{h}")
                for h in range(NH)
            ]
            for d in range(ND):
                tw = wpool.tile([P, NH * P], F32R, tag="tw", bufs=6,
                                name=f"tw{B}{which}{d}")
                nc.sync.dma_start(tw[:], w_dram[d * P : (d + 1) * P, :])
                for h in range(NH):
                    nc.tensor.matmul(
                        psqk[h][:],
                        tw[:, h * P : (h + 1) * P],
                        xT[:, d * BL : (d + 1) * BL],
                        start=(d == 0),
                        stop=(d == ND - 1),
                    )
            for h in range(NH):
                tsw = wpool.tile([P, BL], F32, tag="ropetmp", bufs=3)
                nc.vector.stream_shuffle(tsw[:], psqk[h][:], _SWAP)
                tt2 = wpool.tile([P, BL], F32, tag="ropetmp", bufs=3)
                nc.vector.tensor_mul(tt2[:], tsw[:], t_sig[:])
                tt3 = wpool.tile([P, BL], F32, tag="ropetmp", bufs=3)
                nc.vector.tensor_mul(tt3[:], psqk[h][:], t_cos[:])
                if which == "q":
                    out_sl = qTs[:, h * BL : (h + 1) * BL]
                else:
                    out_sl = kT[:, h * L + l0 : h * L + l0 + BL]
                nc.vector.tensor_add(out_sl, tt3[:], tt2[:])

        def emit_sexp(h, j0):
            off = j0 - NLT * B
            r0 = 0 if off < 0 else min(off, 2) * P
            n = BL - r0
            pss = ps.tile([P, BL], F32, tag="ps", name=f"pss{B}{h}{j0}")
            diag = off >= 0
            if diag:
                # seed psum with the causal mask, accumulate scores on top
                c0 = P - (off * P - r0)
                nc.tensor.matmul(
                    pss[:, r0:BL],
                    t_id[:],
                    t_mask[:, c0 : c0 + n],
                    start=True,
                    stop=False,
                )
            nc.tensor.matmul(
                pss[:, r0:BL],
                kT[:, h * L + j0 * P : h * L + (j0 + 1) * P],
                qTs[:, h * BL + r0 : (h + 1) * BL],
                start=not diag,
                stop=True,
            )
            at = wpool.tile([P, BL], F32R, tag="at", bufs=4,
                            name=f"at{B}{h}{j0}")
            nc.scalar.activation(at[:, r0:BL], pss[:, r0:BL], EXP)
            return at, r0

        # heads 0-1's first score tiles ride ahead of the v projection so
        # the exp latency hides under the v matmuls (their keys are from
        # earlier blocks, so only q/k rope gates them).
        early_at = {}
        if B > 0:
            for eh in range(2):
                for j0 in range(2):
                    early_at[(eh, j0)] = emit_sexp(eh, j0)

        # ---- v projection ----
        psv = [
            ps.tile([P, NH * P], F32, tag="ps", name=f"psv{B}{lt}")
            for lt in range(NLT)
        ]
        for d in range(ND):
            tw = wpool.tile([P, NH * P], F32R, tag="tw", bufs=6,
                            name=f"twv{B}{d}")
            nc.sync.dma_start(tw[:], wv_d[d * P : (d + 1) * P, :])
            for lt in range(NLT):
                nc.tensor.matmul(
                    psv[lt][:],
                    xT[:, d * BL + lt * P : d * BL + (lt + 1) * P],
                    tw[:],
                    start=(d == 0),
                    stop=(d == ND - 1),
                )
        for lt in range(NLT):
            j0 = NLT * B + lt
            nc.vector.tensor_copy(vv[:, j0 * NH * P : (j0 + 1) * NH * P], psv[lt][:])

        # ---- causal attention for this query block ----
        # diag key chunk at offset `off` only contributes to queries
        # i >= off*P; mm range starts at r0 = min(off, 2)*P (f32r needs a
        # moving dim >= 256 for full rate).
        nk = NLT * B + NLT  # valid key chunks
        oTbs = [
            wpool.tile([P, BL], F32R, tag="oTb", bufs=6, name=f"oTb{B}{h}")
            for h in range(NH)
        ]
        # prefetch first Wo tile group so the Wo phase isn't DMA-bound
        two0 = wpool.tile([P, NH, BL], F32R, tag="two", bufs=3, name=f"two{B}0")
        nc.gpsimd.dma_start(two0[:], wo_d[:, :, 0:BL])
        for h in range(NH):
            pso = ps.tile([P, BL], F32, tag="ps", name=f"pso{B}{h}")
            prs = ps.tile([1, BL], F32, tag="ps", name=f"prs{B}{h}")
            for j0 in range(nk):
                if (h, j0) in early_at:
                    at, r0 = early_at[(h, j0)]
                else:
                    at, r0 = emit_sexp(h, j0)
                nc.tensor.matmul(
                    pso[:, r0:BL],
                    vv[:, j0 * NH * P + h * P : j0 * NH * P + (h + 1) * P],
                    at[:, r0:BL],
                    start=(j0 == 0),
                    stop=(j0 == nk - 1),
                )
                nc.tensor.matmul(
                    prs[:, r0:BL],
                    t_ones[:],
                    at[:, r0:BL],
                    start=(j0 == 0),
                    stop=(j0 == nk - 1),
                )
            # per-head softmax denominator chain (pipelined with the next
            # head's attention): spread 512 sums over 128 partitions so the
            # iterative reciprocal is cheap; rebroadcast via one DMA into
            # partitions {0,32,64,96} plus a stream_shuffle.
            dmae = nc.sync if h == NH - 1 else nc.gpsimd
            rtmp = wpool.tile([1, BL], F32, tag="rtmp", bufs=2)
            nc.scalar.copy(rtmp[:], prs[:])
            rsp = wpool.tile([P, NLT], F32, tag="rsp")
            dmae.dma_start(
                rsp[:], rtmp[0:1, :].rearrange("one (p c) -> one p c", c=NLT)
            )
            nc.vector.reciprocal(rsp[:], rsp[:])
            rinv1 = wpool.tile([1, BL], F32, tag="rtmp", bufs=2,
                               name=f"rinv1{B}{h}")
            dmae.dma_start(
                rinv1[0:1, :].rearrange("one (p c) -> one p c", c=NLT), rsp[:]
            )
            rb = wpool.tile([P, BL], F32, tag="rb", bufs=1, name=f"rb{B}{h}")
            nc.gpsimd.partition_broadcast(rb[:], rinv1[:])
            nc.vector.tensor_mul(oTbs[h][:], pso[:], rb[:])

        # one block ahead: transposes ride the denominator-chain stall
        if B + 1 < NB:
            xT_next = emit_transpose(B + 1)

        # ---- partial output projection for this block's rows ----
        for Db in range(NB):
            if Db == 0:
                two = two0
            else:
                two = wpool.tile([P, NH, BL], F32R, tag="two", bufs=3,
                                 name=f"two{B}{Db}")
                nc.sync.dma_start(
                    two[:], wo_d[:, :, Db * BL : (Db + 1) * BL]
                )
            psys = [
                ps.tile([P, BL], F32, tag="ps", name=f"psy{B}{Db}{lt}")
                for lt in range(NLT)
            ]
            if Db == 0:
                # h-outer emission: 12 mms of runway before oTb[3] (whose
                # reciprocal chain trails the last AV matmul) is needed.
                for h in range(NH):
                    for lt in range(NLT):
                        nc.tensor.matmul(
                            psys[lt][:],
                            oTbs[h][:, lt * P : (lt + 1) * P],
                            two[:, h, :],
                            start=(h == 0),
                            stop=(h == NH - 1),
                        )
            else:
                for lt in range(NLT):
                    for h in range(NH):
                        nc.tensor.matmul(
                            psys[lt][:],
                            oTbs[h][:, lt * P : (lt + 1) * P],
                            two[:, h, :],
                            start=(h == 0),
                            stop=(h == NH - 1),
                        )
            for lt in range(NLT):
                ye = wpool.tile([P, BL], F32, tag="ye", bufs=2,
                                name=f"ye{B}{Db}{lt}")
                nc.vector.tensor_copy(ye[:], psys[lt][:])
                nc.scalar.dma_start(
                    y_d[l0 + lt * P : l0 + (lt + 1) * P, Db * BL : (Db + 1) * BL],
                    ye[:],
                )


_NC_CACHE = None


def _get_program():
    global _NC_CACHE
    if _NC_CACHE is None:
        _NC_CACHE = build_program()
    return _NC_CACHE


def _host_tables():
    hd = P  # head dim
    i = np.arange(hd // 2, dtype=np.float64)
    invf = THETA ** (-2.0 * i / hd)  # [64]
    t = np.arange(L, dtype=np.float64)
    ang = np.outer(invf, t)  # [64, L]
    cos = np.cos(ang)
    sin = np.sin(ang)
    cosT = np.repeat(cos, 2, axis=0).astype(np.float32)  # [128, L]
    sigT = np.empty((P, L), dtype=np.float32)
    sigT[0::2] = -sin
    sigT[1::2] = sin

    # mask[j, cc] = 0 iff (cc - P) >= j : column cc maps to query offset
    # (cc - P) relative to the key chunk start; cc < P rows are all masked.
    j = np.arange(P)[:, None]
    cc = np.arange(5 * P)[None, :]
    masks = np.where(cc - P >= j, 0.0, NEG).astype(np.float32)
    return cosT, sigT, masks


def kernel(x, Wq, Wk, Wv, Wo):
    x = np.asarray(x, dtype=np.float32)
    Wq = np.asarray(Wq, dtype=np.float32)
    Wk = np.asarray(Wk, dtype=np.float32)
    Wv = np.asarray(Wv, dtype=np.float32)
    Wo = np.asarray(Wo, dtype=np.float32)
    Bsz = x.shape[0]

    nc = _get_program()
    cosT, sigT, masks = _host_tables()
    iden = np.eye(P, dtype=np.float32)
    ones = np.ones((P, 1), dtype=np.float32)
    scale = 1.0 / np.sqrt(float(P))

    in_maps = []
    for c in range(8):
        b = c // 4
        g = c % 4
        hs = slice(NH * g, NH * g + NH)
        wo_c = Wo[NH * P * g : NH * P * (g + 1), :]  # [NH*P, D]
        wo_r = np.ascontiguousarray(
            wo_c.reshape(NH, P, D).transpose(1, 0, 2)
        )  # [P, NH, D]
        in_maps.append(
            {
                "x": np.ascontiguousarray(x[b]),
                "wq": np.ascontiguousarray(
                    Wq[:, hs, :].reshape(D, NH * P) * scale
                ),
                "wk": np.ascontiguousarray(Wk[:, hs, :].reshape(D, NH * P)),
                "wv": np.ascontiguousarray(Wv[:, hs, :].reshape(D, NH * P)),
                "wo": wo_r,
                "cosT": cosT,
                "sigT": sigT,
                "masks": masks,
                "iden": iden,
                "ones": ones,
            }
        )

    res = run_bass_kernel_spmd(nc, in_maps, list(range(8)))
    y = np.zeros((Bsz, L, D), dtype=np.float32)
    for c in range(8):
        y[c // 4] += res.results[c]["y"]
    return y


# revision 48
# speedup vs baseline: 1.0293x; 1.0293x over previous
"""Causal multi-head attention (B=2, L=2048, D=2048, H=16, rope theta=5e5)
on 8 Trainium2 NeuronCores.

Sharding: core c handles batch b = c//4 and heads 4*(c%4) .. 4*(c%4)+3.
Each core computes q/k/v projections for its 4 heads, rope, causal
attention, and a partial output projection y_part = attn_out @ Wo[rows].
Host sums the 4 partials per batch (row-parallel all-reduce) to produce y.

All matmuls run in float32r (TF32-like, 1 cycle/row on the PE) with fp32
accumulation. Softmax skips the max-subtraction (scores are ~N(0,1);
max over 134M samples < 7, exp is safe in fp32).
"""
import os
import sys

sys.path.insert(0, "/opt/trn_rl_repo")

import contextlib

import numpy as np

import concourse.bass as bass
import concourse.tile as tile
from concourse import bacc, mybir
from concourse.bass_utils import run_bass_kernel_spmd

F32 = mybir.dt.float32
F32R = mybir.dt.float32r
EXP = mybir.ActivationFunctionType.Exp

P = 128          # partitions / head dim / tile edge
L = 2048         # sequence length
D = 2048         # model dim
NH = 4           # heads per core
BL = 512         # query block (l-block) size
NB = L // BL     # 4 l-blocks
ND = D // P      # 16 d chunks
NDG = 4          # d chunks per weight DMA group
NLT = BL // P    # 4 l-tiles per block
THETA = 500000.0
NEG = -1.0e30

_SWAP = []
for _i in range(16):
    _SWAP += [2 * _i + 1, 2 * _i]


def build_program(nrep=1):
    nc = bacc.Bacc("TRN2", target_bir_lowering=False, debug=False, num_devices=8)

    x_d = nc.dram_tensor("x", [L, D], F32R, kind="ExternalInput").ap()
    wq_d = nc.dram_tensor("wq", [D, NH * P], F32R, kind="ExternalInput").ap()
    wk_d = nc.dram_tensor("wk", [D, NH * P], F32R, kind="ExternalInput").ap()
    wv_d = nc.dram_tensor("wv", [D, NH * P], F32R, kind="ExternalInput").ap()
    # wo pre-permuted on host to [P, NH, D]: wo_r[p, h, :] = Wo[h*P + p, :]
    wo_d = nc.dram_tensor("wo", [P, NH, D], F32R, kind="ExternalInput").ap()
    cos_d = nc.dram_tensor("cosT", [P, L], F32, kind="ExternalInput").ap()
    sig_d = nc.dram_tensor("sigT", [P, L], F32, kind="ExternalInput").ap()
    mask_d = nc.dram_tensor("masks", [P, 5 * P], F32R, kind="ExternalInput").ap()
    id_d = nc.dram_tensor("iden", [P, P], F32R, kind="ExternalInput").ap()
    ones_d = nc.dram_tensor("ones", [P, 1], F32R, kind="ExternalInput").ap()
    y_d = nc.dram_tensor("y", [L, D], F32, kind="ExternalOutput").ap()

    with tile.TileContext(nc) as tc:
        with (
            tc.tile_pool(name="const", bufs=1) as cpool,
            tc.tile_pool(name="persist", bufs=1) as ppool,
            tc.tile_pool(name="work", bufs=2) as wpool,
            tc.tile_pool(name="ps", bufs=8, space="PSUM") as ps,
        ):
            t_id = cpool.tile([P, P], F32R, tag="t_id")
            t_ones = cpool.tile([P, 1], F32R, tag="t_ones")
            t_mask = cpool.tile([P, 5 * P], F32R, tag="t_mask")
            nc.sync.dma_start(t_id[:], id_d[:])
            nc.sync.dma_start(t_ones[:], ones_d[:])
            nc.sync.dma_start(t_mask[:], mask_d[:])

            # persistent full-L tensors (per head slices along free axis)
            kT = ppool.tile([P, NH * L], F32R, tag="kT")     # [dh, h*L + l]
            vv = ppool.tile([P, ND * NH * P], F32R, tag="vv")  # [l%128, j0*512 + h*128 + dh]
            qTs = ppool.tile([P, NH * BL], F32R, tag="qTs")  # current block [dh, h*BL + l]

            rep_ctx = tc.For_i(0, nrep, 1) if nrep > 1 else contextlib.nullcontext()
            with rep_ctx:
                _build_body(nc, tc, locals())
    nc.compile()
    return nc


def _build_body(nc, tc, env):
    t_id, t_ones, t_mask = env["t_id"], env["t_ones"], env["t_mask"]
    kT, vv, qTs = env["kT"], env["vv"], env["qTs"]
    wpool, ps = env["wpool"], env["ps"]
    x_d, wq_d, wk_d, wv_d, wo_d = (
        env["x_d"], env["wq_d"], env["wk_d"], env["wv_d"], env["wo_d"]
    )
    cos_d, sig_d, y_d = env["cos_d"], env["sig_d"], env["y_d"]

    def emit_transpose(B):
        """x[B-block rows] -> xT (PE transposes).  Emitted one block ahead so
        the x DMAs get a phase of lead time and the PE transposes fill the
        softmax-denominator stall after attention."""
        l0 = B * BL
        xT = wpool.tile([P, ND * BL], F32R, tag="xT", bufs=1, name=f"xT{B}")
        xTv = xT[:].rearrange("p (dd c) -> p dd c", c=BL)
        for lt in range(NLT):
            xr = wpool.tile([P, D], F32R, tag="xr", bufs=3, name=f"xr{B}{lt}")
            if B == 0 and lt == 0:
                for q4 in range(4):
                    nc.sync.dma_start(
                        xr[:, q4 * BL : (q4 + 1) * BL],
                        x_d[l0 : l0 + P, q4 * BL : (q4 + 1) * BL],
                    )
            else:
                nc.sync.dma_start(xr[:], x_d[l0 + lt * P : l0 + (lt + 1) * P, :])
            for dg in range(4):
                pt = ps.tile([P, 4 * P], F32R, tag="ps")
                for i in range(4):
                    d = dg * 4 + i
                    nc.tensor.transpose(
                        pt[:, i * P : (i + 1) * P],
                        xr[:, d * P : (d + 1) * P],
                        t_id[:],
                    )
                nc.scalar.copy(
                    xTv[:, dg * 4 : (dg + 1) * 4, lt * P : (lt + 1) * P],
                    pt[:].rearrange("p (i c) -> p i c", c=P),
                )
        return xT

    xT_next = emit_transpose(0)
    for B in range(NB):
        l0 = B * BL
        xT = xT_next
        # rope tables for this block (gpsimd queue: keep SP free for weights)
        t_cos = wpool.tile([P, BL], F32, tag="t_cos", bufs=1)
        t_sig = wpool.tile([P, BL], F32, tag="t_sig", bufs=1)
        nc.gpsimd.dma_start(t_cos[:], cos_d[:, l0 : l0 + BL])
        nc.gpsimd.dma_start(t_sig[:], sig_d[:, l0 : l0 + BL])

        # ---- q / k projections + rope ----
        for which, w_dram in (("q", wq_d), ("k", wk_d)):
            psqk = [
                ps.tile([P, BL], F32, tag="ps", name=f"psqk{B}{which}{h}")
                for h in range(NH)
            ]
            for d in range(ND):
                tw = wpool.tile([P, NH * P], F32R, tag="tw", bufs=6,
                                name=f"tw{B}{which}{d}")
                nc.sync.dma_start(tw[:], w_dram[d * P : (d + 1) * P, :])
                for h in range(NH):
                    nc.tensor.matmul(
                        psqk[h][:],
                        tw[:, h * P : (h + 1) * P],
                        xT[:, d * BL : (d + 1) * BL],
                        start=(d == 0),
                        stop=(d == ND - 1),
                    )
            for h in range(NH):
                tsw = wpool.tile([P, BL], F32, tag="ropetmp", bufs=3)
                nc.vector.stream_shuffle(tsw[:], psqk[h][:], _SWAP)
                tt2 = wpool.tile([P, BL], F32, tag="ropetmp", bufs=3)
                nc.vector.tensor_mul(tt2[:], tsw[:], t_sig[:])
                tt3 = wpool.tile([P, BL], F32, tag="ropetmp", bufs=3)
                nc.vector.tensor_mul(tt3[:], psqk[h][:], t_cos[:])
                if which == "q":
                    out_sl = qTs[:, h * BL : (h + 1) * BL]
                else:
                    out_sl = kT[:, h * L + l0 : h * L + l0 + BL]
                nc.vector.tensor_add(out_sl, tt3[:], tt2[:])

        def emit_sexp(h, j0):
            off = j0 - NLT * B
            r0 = 0 if off < 0 else min(off, 2) * P
            n = BL - r0
            pss = ps.tile([P, BL], F32, tag="ps", name=f"pss{B}{h}{j0}")
            diag = off >= 0
            if diag:
                # seed psum with the causal mask, accumulate scores on top
                c0 = P - (off * P - r0)
                nc.tensor.matmul(
                    pss[:, r0:BL],
                    t_id[:],
                    t_mask[:, c0 : c0 + n],
                    start=True,
                    stop=False,
                )
            nc.tensor.matmul(
                pss[:, r0:BL],
                kT[:, h * L + j0 * P : h * L + (j0 + 1) * P],
                qTs[:, h * BL + r0 : (h + 1) * BL],
                start=not diag,
                stop=True,
            )
            at = wpool.tile([P, BL], F32R, tag="at", bufs=3,
                            name=f"at{B}{h}{j0}")
            nc.scalar.activation(at[:, r0:BL], pss[:, r0:BL], EXP)
            return at, r0

        # head 0's first score tiles ride ahead of the v projection so the
        # exp latency hides under the v matmuls (their keys are from earlier
        # blocks, so only q/k rope gates them).
        early_at = {}
        if B > 0:
            for j0 in range(2):
                early_at[(0, j0)] = emit_sexp(0, j0)

        # ---- v projection ----
        psv = [
            ps.tile([P, NH * P], F32, tag="ps", name=f"psv{B}{lt}")
            for lt in range(NLT)
        ]
        for d in range(ND):
            tw = wpool.tile([P, NH * P], F32R, tag="tw", bufs=6,
                            name=f"twv{B}{d}")
            nc.sync.dma_start(tw[:], wv_d[d * P : (d + 1) * P, :])
            for lt in range(NLT):
                nc.tensor.matmul(
                    psv[lt][:],
                    xT[:, d * BL + lt * P : d * BL + (lt + 1) * P],
                    tw[:],
                    start=(d == 0),
                    stop=(d == ND - 1),
                )
        for lt in range(NLT):
            j0 = NLT * B + lt
            nc.vector.tensor_copy(vv[:, j0 * NH * P : (j0 + 1) * NH * P], psv[lt][:])

        # ---- causal attention for this query block ----
        # diag key chunk at offset `off` only contributes to queries
        # i >= off*P; mm range starts at r0 = min(off, 2)*P (f32r needs a
        # moving dim >= 256 for full rate).
        nk = NLT * B + NLT  # valid key chunks
        oTbs = [
            wpool.tile([P, BL], F32R, tag="oTb", bufs=6, name=f"oTb{B}{h}")
            for h in range(NH)
        ]
        # prefetch first Wo tile group so the Wo phase isn't DMA-bound
        two0 = wpool.tile([P, NH, BL], F32R, tag="two", bufs=3, name=f"two{B}0")
        nc.gpsimd.dma_start(two0[:], wo_d[:, :, 0:BL])
        for h in range(NH):
            pso = ps.tile([P, BL], F32, tag="ps", name=f"pso{B}{h}")
            prs = ps.tile([1, BL], F32, tag="ps", name=f"prs{B}{h}")
            for j0 in range(nk):
                if (h, j0) in early_at:
                    at, r0 = early_at[(h, j0)]
                else:
                    at, r0 = emit_sexp(h, j0)
                nc.tensor.matmul(
                    pso[:, r0:BL],
                    vv[:, j0 * NH * P + h * P : j0 * NH * P + (h + 1) * P],
                    at[:, r0:BL],
                    start=(j0 == 0),
                    stop=(j0 == nk - 1),
                )
                nc.tensor.matmul(
                    prs[:, r0:BL],
                    t_ones[:],
                    at[:, r0:BL],
                    start=(j0 == 0),
                    stop=(j0 == nk - 1),
                )
            # per-head softmax denominator chain (pipelined with the next
            # head's attention): spread 512 sums over 128 partitions so the
            # iterative reciprocal is cheap; rebroadcast via one DMA into
            # partitions {0,32,64,96} plus a stream_shuffle.
            dmae = nc.sync if h == NH - 1 else nc.gpsimd
            rtmp = wpool.tile([1, BL], F32, tag="rtmp", bufs=2)
            nc.scalar.copy(rtmp[:], prs[:])
            rsp = wpool.tile([P, NLT], F32, tag="rsp")
            dmae.dma_start(
                rsp[:], rtmp[0:1, :].rearrange("one (p c) -> one p c", c=NLT)
            )
            nc.vector.reciprocal(rsp[:], rsp[:])
            rinv1 = wpool.tile([1, BL], F32, tag="rtmp", bufs=2,
                               name=f"rinv1{B}{h}")
            dmae.dma_start(
                rinv1[0:1, :].rearrange("one (p c) -> one p c", c=NLT), rsp[:]
            )
            rb = wpool.tile([P, BL], F32, tag="rb", bufs=2, name=f"rb{B}{h}")
            nc.gpsimd.partition_broadcast(rb[:], rinv1[:])
            nc.vector.tensor_mul(oTbs[h][:], pso[:], rb[:])

        # one block ahead: transposes ride the denominator-chain stall
        if B + 1 < NB:
            xT_next = emit_transpose(B + 1)

        # ---- partial output projection for this block's rows ----
        for Db in range(NB):
            if Db == 0:
                two = two0
            else:
                two = wpool.tile([P, NH, BL], F32R, tag="two", bufs=3,
                                 name=f"two{B}{Db}")
                nc.sync.dma_start(
                    two[:], wo_d[:, :, Db * BL : (Db + 1) * BL]
                )
            psys = [
                ps.tile([P, BL], F32, tag="ps", name=f"psy{B}{Db}{lt}")
                for lt in range(NLT)
            ]
            if Db == 0:
                # h-outer emission: 12 mms of runway before oTb[3] (whose
                # reciprocal chain trails the last AV matmul) is needed.
                for h in range(NH):
                    for lt in range(NLT):
                        nc.tensor.matmul(
                            psys[lt][:],
                            oTbs[h][:, lt * P : (lt + 1) * P],
                            two[:, h, :],
                            start=(h == 0),
                            stop=(h == NH - 1),
                        )
            else:
                for lt in range(NLT):
                    for h in range(NH):
                        nc.tensor.matmul(
                            psys[lt][:],
                            oTbs[h][:, lt * P : (lt + 1) * P],
                            two[:, h, :],
                            start=(h == 0),
                            stop=(h == NH - 1),
                        )
            for lt in range(NLT):
                ye = wpool.tile([P, BL], F32, tag="ye", bufs=2,
                                name=f"ye{B}{Db}{lt}")
                nc.vector.tensor_copy(ye[:], psys[lt][:])
                nc.scalar.dma_start(
                    y_d[l0 + lt * P : l0 + (lt + 1) * P, Db * BL : (Db + 1) * BL],
                    ye[:],
                )


_NC_CACHE = None


def _get_program():
    global _NC_CACHE
    if _NC_CACHE is None:
        _NC_CACHE = build_program()
    return _NC_CACHE


def _host_tables():
    hd = P  # head dim
    i = np.arange(hd // 2, dtype=np.float64)
    invf = THETA ** (-2.0 * i / hd)  # [64]
    t = np.arange(L, dtype=np.float64)
    ang = np.outer(invf, t)  # [64, L]
    cos = np.cos(ang)
    sin = np.sin(ang)
    cosT = np.repeat(cos, 2, axis=0).astype(np.float32)  # [128, L]
    sigT = np.empty((P, L), dtype=np.float32)
    sigT[0::2] = -sin
    sigT[1::2] = sin

    # mask[j, cc] = 0 iff (cc - P) >= j : column cc maps to query offset
    # (cc - P) relative to the key chunk start; cc < P rows are all masked.
    j = np.arange(P)[:, None]
    cc = np.arange(5 * P)[None, :]
    masks = np.where(cc - P >= j, 0.0, NEG).astype(np.float32)
    return cosT, sigT, masks


def kernel(x, Wq, Wk, Wv, Wo):
    x = np.asarray(x, dtype=np.float32)
    Wq = np.asarray(Wq, dtype=np.float32)
    Wk = np.asarray(Wk, dtype=np.float32)
    Wv = np.asarray(Wv, dtype=np.float32)
    Wo = np.asarray(Wo, dtype=np.float32)
    Bsz = x.shape[0]

    nc = _get_program()
    cosT, sigT, masks = _host_tables()
    iden = np.eye(P, dtype=np.float32)
    ones = np.ones((P, 1), dtype=np.float32)
    scale = 1.0 / np.sqrt(float(P))

    in_maps = []
    for c in range(8):
        b = c // 4
        g = c % 4
        hs = slice(NH * g, NH * g + NH)
        wo_c = Wo[NH * P * g : NH * P * (g + 1), :]  # [NH*P, D]
        wo_r = np.ascontiguousarray(
            wo_c.reshape(NH, P, D).transpose(1, 0, 2)
        )  # [P, NH, D]
        in_maps.append(
            {
                "x": np.ascontiguousarray(x[b]),
                "wq": np.ascontiguousarray(
                    Wq[:, hs, :].reshape(D, NH * P) * scale
                ),
                "wk": np.ascontiguousarray(Wk[:, hs, :].reshape(D, NH * P)),
                "wv": np.ascontiguousarray(Wv[:, hs, :].reshape(D, NH * P)),
                "wo": wo_r,
                "cosT": cosT,
                "sigT": sigT,
                "masks": masks,
                "iden": iden,
                "ones": ones,
            }
        )

    res = run_bass_kernel_spmd(nc, in_maps, list(range(8)))
    y = np.zeros((Bsz, L, D), dtype=np.float32)
    for c in range(8):
        y[c // 4] += res.results[c]["y"]
    return y
